# revision 19
# baseline (speedup 1.0000x reference)
"""Trainium2 kernel for CompactBilinearLayer (count-sketch bilinear pooling).

Math: reference computes y = l2norm(signed_sqrt(sum_hw Re IFFT(FFT(x@M1)*FFT(x@M2)))).
Since M1/M2 are count-sketch matrices (one +-1 per row), FFT(x@M1) == x @ A1 with
A1[c,k] = s1[c] * exp(-2pi i h1[c] k / P) — a dense [512, K] matrix computable on the
host from M1 in O(C*K). The IFFT is linear, so the spatial sum moves before it.
Hermitian symmetry means only k = 0..4096 are needed.  Per core (4 batch elements,
784 spatial positions — fully batch-local, no collectives):
  A: P1/P2 projections  = A^T @ x^T          (4 real matmuls, k on partitions)
  B: S[k,b] = sum_t (P1*P2) per batch        (complex product + segmented reduce)
  C: IFFT via two-step factorization n=64q+s: U/V twiddle (DVE) + matmul over k%128
  D: signed sqrt + per-batch L2 norm + store
"""
import numpy as np

P = 8192
C = 512
FT = 33            # frequency tiles of 128 -> 4224 slots >= 4097
NSLOT = FT * 128
NCORES = 8
BPC = 4            # batch elems per core
HW = 196           # spatial positions per batch elem
T = BPC * HW       # 784 positions per core
B = 32

_CACHE = {}


def _build_program():
    import concourse.bass as bass
    import concourse.tile as tile
    from concourse import bacc, mybir

    f32 = mybir.dt.float32
    f32r = mybir.dt.float32r
    nc = bacc.Bacc("TRN2", target_bir_lowering=False, debug=False,
                   num_devices=NCORES)

    # stage-A operands come pre-split (hi/lo, each RNE-rounded to fp32r's
    # 11 mantissa bits on the host); 3 full-rate fp32r matmuls give exact
    # fp32-quality products at 3/4 the PE cost of native fp32 (4 cyc/row).
    ah_d = nc.dram_tensor("ah", [FT, C, 512], f32r, kind="ExternalInput").ap()
    al_d = nc.dram_tensor("al", [FT, C, 512], f32r, kind="ExternalInput").ap()
    xh_d = nc.dram_tensor("xh", [C, T], f32r, kind="ExternalInput").ap()
    xl_d = nc.dram_tensor("xl", [C, T], f32r, kind="ExternalInput").ap()
    cphi_d = nc.dram_tensor("cphi", [FT, 128, 64], f32, kind="ExternalInput").ap()
    sphi_d = nc.dram_tensor("sphi", [FT, 128, 64], f32, kind="ExternalInput").ap()
    cosah_d = nc.dram_tensor("cosah", [128, 128], f32r, kind="ExternalInput").ap()
    cosal_d = nc.dram_tensor("cosal", [128, 128], f32r, kind="ExternalInput").ap()
    nsinah_d = nc.dram_tensor("nsinah", [128, 128], f32r, kind="ExternalInput").ap()
    nsinal_d = nc.dram_tensor("nsinal", [128, 128], f32r, kind="ExternalInput").ap()
    y_d = nc.dram_tensor("y", [BPC, P], f32, kind="ExternalOutput").ap()

    mult = mybir.AluOpType.mult
    add = mybir.AluOpType.add
    Act = mybir.ActivationFunctionType

    with tile.TileContext(nc) as tc:
        with (
            tc.tile_pool(name="const", bufs=1) as const,
            tc.tile_pool(name="apool", bufs=3) as apool,
            tc.tile_pool(name="ps", bufs=1, space="PSUM") as pspool,
            tc.tile_pool(name="scr", bufs=3) as scr,
            tc.tile_pool(name="uv", bufs=3) as uvpool,
        ):
            xh_sb = const.tile([128, 4, T], f32r)
            nc.sync.dma_start(xh_sb[:], xh_d.rearrange("(ck p) t -> p ck t", p=128))
            xl_sb = const.tile([128, 4, T], f32r)
            nc.sync.dma_start(xl_sb[:], xl_d.rearrange("(ck p) t -> p ck t", p=128))
            cphi_sb = const.tile([128, FT, 64], f32)
            nc.sync.dma_start(cphi_sb[:], cphi_d.rearrange("kt p s -> p kt s"))
            sphi_sb = const.tile([128, FT, 64], f32)
            nc.sync.dma_start(sphi_sb[:], sphi_d.rearrange("kt p s -> p kt s"))
            cosah_sb = const.tile([128, 128], f32r)
            nc.sync.dma_start(cosah_sb[:], cosah_d)
            cosal_sb = const.tile([128, 128], f32r)
            nc.sync.dma_start(cosal_sb[:], cosal_d)
            nsinah_sb = const.tile([128, 128], f32r)
            nc.sync.dma_start(nsinah_sb[:], nsinah_d)
            nsinal_sb = const.tile([128, 128], f32r)
            nc.sync.dma_start(nsinal_sb[:], nsinal_d)
            ones_sb = const.tile([128, 1], f32)
            nc.vector.memset(ones_sb[:], 1.0)
            sre_sb = const.tile([128, FT * 4], f32)
            sim_sb = const.tile([128, FT * 4], f32)

            # ---- stage A+B: projections, complex product, spatial reduce ----
            for ft in range(FT):
                ah_t = apool.tile([128, 4, 512], f32r, tag="ah")
                nc.sync.dma_start(
                    ah_t[:], ah_d[ft].rearrange("(ck p) m -> p ck m", p=128)
                )
                al_t = apool.tile([128, 4, 512], f32r, tag="al")
                nc.sync.dma_start(
                    al_t[:], al_d[ft].rearrange("(ck p) m -> p ck m", p=128)
                )
                ps = [
                    pspool.tile([128, T], f32, tag=f"p{m}", name=f"ps{m}_{ft}")
                    for m in range(4)
                ]
                for m in range(4):
                    msl = slice(m * 128, (m + 1) * 128)
                    for ck in range(4):
                        for c0, cn in ((0, 512), (512, T - 512)):
                            terms = (
                                (ah_t[:, ck, msl], xh_sb[:, ck, c0:c0 + cn]),
                                (ah_t[:, ck, msl], xl_sb[:, ck, c0:c0 + cn]),
                                (al_t[:, ck, msl], xh_sb[:, ck, c0:c0 + cn]),
                            )
                            for ti, (lhs, rhs) in enumerate(terms):
                                nc.tensor.matmul(
                                    ps[m][:, c0:c0 + cn],
                                    lhs,
                                    rhs,
                                    start=(ck == 0 and ti == 0),
                                    stop=(ck == 3 and ti == 2),
                                )
                # DVE reads at most one PSUM operand; stage the A2 pair in SBUF
                p2sb = scr.tile([128, T], f32, tag="p2sb")
                p3sb = scr.tile([128, T], f32, tag="p3sb")
                nc.scalar.activation(p2sb[:], ps[2][:], Act.Copy)
                nc.scalar.activation(p3sb[:], ps[3][:], Act.Copy)
                operands = ((ps[0], p2sb), (ps[1], p3sb), (ps[0], p3sb), (ps[1], p2sb))
                red = []
                for i, (pa, pb) in enumerate(operands):
                    prod = scr.tile([128, T], f32, tag=f"prod{i}",
                                    name=f"prod{i}_{ft}")
                    nc.vector.tensor_tensor(prod[:], pa[:], pb[:], op=mult)
                    r = scr.tile([128, BPC], f32, tag=f"red{i}",
                                 name=f"red{i}_{ft}")
                    nc.vector.reduce_sum(
                        out=r[:],
                        in_=prod[:].rearrange("p (b t) -> p b t", b=BPC),
                        axis=mybir.AxisListType.X,
                    )
                    red.append(r)
                sblk = slice(ft * 4, (ft + 1) * 4)
                nc.vector.tensor_sub(sre_sb[:, sblk], red[0][:], red[1][:])
                nc.vector.tensor_add(sim_sb[:, sblk], red[2][:], red[3][:])

            # ---- stage C: twiddle + IFFT matmul over k mod 128 ----
            psy = pspool.tile([128, BPC * 64], f32, tag="p0")
            for kt in range(FT):
                cph = cphi_sb[:, kt, :][:, None, :].broadcast_to([128, BPC, 64])
                sph = sphi_sb[:, kt, :][:, None, :].broadcast_to([128, BPC, 64])
                sre = sre_sb[:, kt * 4:(kt + 1) * 4][:, :, None].broadcast_to(
                    [128, BPC, 64])
                sim = sim_sb[:, kt * 4:(kt + 1) * 4][:, :, None].broadcast_to(
                    [128, BPC, 64])
                u1 = uvpool.tile([128, BPC, 64], f32, tag="u1")
                u2 = uvpool.tile([128, BPC, 64], f32, tag="u2")
                uu = uvpool.tile([128, BPC * 64], f32, tag="uu")
                v1 = uvpool.tile([128, BPC, 64], f32, tag="v1")
                v2 = uvpool.tile([128, BPC, 64], f32, tag="v2")
                vv = uvpool.tile([128, BPC * 64], f32, tag="vv")
                # U chain on DVE, V chain on (otherwise idle) GpSimd
                nc.vector.tensor_tensor(u1[:], cph, sre, op=mult)
                nc.vector.tensor_tensor(u2[:], sph, sim, op=mult)
                nc.vector.tensor_sub(
                    uu[:].rearrange("p (b s) -> p b s", b=BPC), u1[:], u2[:])
                nc.gpsimd.tensor_tensor(v1[:], sph, sre, op=mult)
                nc.gpsimd.tensor_tensor(v2[:], cph, sim, op=mult)
                nc.gpsimd.tensor_add(
                    vv[:].rearrange("p (b s) -> p b s", b=BPC), v1[:], v2[:])
                # split U/V into fp32r hi/lo for full-rate matmuls
                uuh = uvpool.tile([128, BPC * 64], f32r, tag="uuh")
                uul32 = uvpool.tile([128, BPC * 64], f32, tag="uul32")
                uul = uvpool.tile([128, BPC * 64], f32r, tag="uul")
                nc.vector.tensor_copy(uuh[:], uu[:])
                nc.vector.tensor_sub(uul32[:], uu[:], uuh[:].bitcast(f32))
                nc.vector.tensor_copy(uul[:], uul32[:])
                vvh = uvpool.tile([128, BPC * 64], f32r, tag="vvh")
                vvl32 = uvpool.tile([128, BPC * 64], f32, tag="vvl32")
                vvl = uvpool.tile([128, BPC * 64], f32r, tag="vvl")
                nc.gpsimd.tensor_copy(vvh[:], vv[:])
                nc.gpsimd.tensor_sub(vvl32[:], vv[:], vvh[:].bitcast(f32))
                nc.gpsimd.tensor_copy(vvl[:], vvl32[:])
                for ti, (lhs, rhs) in enumerate((
                    (cosah_sb, uuh), (cosah_sb, uul), (cosal_sb, uuh),
                    (nsinah_sb, vvh), (nsinah_sb, vvl), (nsinal_sb, vvh),
                )):
                    nc.tensor.matmul(psy[:], lhs[:], rhs[:],
                                     start=(kt == 0 and ti == 0),
                                     stop=(kt == FT - 1 and ti == 5))

            # ---- stage D: signed sqrt, per-batch l2 norm, store ----
            absy = scr.tile([128, BPC * 64], f32, tag="absy")
            nc.scalar.activation(absy[:], psy[:], Act.Abs)
            sqy = scr.tile([128, BPC * 64], f32, tag="sqy")
            nc.scalar.activation(sqy[:], absy[:], Act.Sqrt)
            sgn = scr.tile([128, BPC * 64], f32, tag="sgn")
            nc.scalar.activation(sgn[:], psy[:], Act.Sign)
            ys = scr.tile([128, BPC * 64], f32, tag="ys")
            nc.vector.tensor_mul(ys[:], sqy[:], sgn[:])

            psn = pspool.tile([128, BPC * 64], f32, tag="p1")
            nc.tensor.matmul(psn[0:1, :], ones_sb[:], absy[:],
                             start=True, stop=True)
            nsq = scr.tile([1, BPC], f32, tag="nsq")
            nc.vector.reduce_sum(
                out=nsq[:],
                in_=psn[0:1, :].rearrange("p (b s) -> p b s", b=BPC),
                axis=mybir.AxisListType.X,
            )
            nc.vector.tensor_scalar_max(nsq[:], nsq[:], 1e-10)
            sqn = scr.tile([1, BPC], f32, tag="sqn")
            nc.scalar.activation(sqn[:], nsq[:], Act.Sqrt)
            invn = scr.tile([1, BPC], f32, tag="invn")
            nc.vector.reciprocal(invn[:], sqn[:])

            onesrow = const.tile([1, 128], f32)
            nc.vector.memset(onesrow[:], 1.0)
            psb = pspool.tile([128, BPC * 64], f32, tag="p2")
            nc.tensor.matmul(psb[:, 0:BPC], onesrow[0:1, :], invn[0:1, :],
                             start=True, stop=True)
            inv_b = psb[:, 0:BPC][:, :, None].broadcast_to([128, BPC, 64])
            fin = scr.tile([128, BPC * 64], f32, tag="fin")
            nc.vector.tensor_tensor(
                fin[:].rearrange("p (b s) -> p b s", b=BPC),
                ys[:].rearrange("p (b s) -> p b s", b=BPC),
                inv_b,
                op=mult,
            )
            for b in range(BPC):
                nc.sync.dma_start(
                    y_d[b].rearrange("(q s) -> q s", q=128),
                    fin[:, b * 64:(b + 1) * 64],
                )

    nc.compile()
    return nc


def _host_prep(x, M1, M2):
    x = np.ascontiguousarray(np.asarray(x, np.float32))
    M1 = np.asarray(M1, np.float32)
    M2 = np.asarray(M2, np.float32)

    h1 = np.argmax(np.abs(M1), axis=1)
    s1 = M1[np.arange(C), h1].astype(np.float64)
    h2 = np.argmax(np.abs(M2), axis=1)
    s2 = M2[np.arange(C), h2].astype(np.float64)

    k = np.arange(NSLOT, dtype=np.float64)
    valid = k <= P // 2
    ang1 = 2 * np.pi * np.outer(h1.astype(np.float64), k) / P
    ang2 = 2 * np.pi * np.outer(h2.astype(np.float64), k) / P
    # a[ft, c, m*128 + j]: m in (A1re, A1im, A2re, A2im), freq = ft*128 + j
    a = np.empty((FT, C, 512), np.float32)
    a1re = (s1[:, None] * np.cos(ang1) * valid).astype(np.float32)
    a1im = (-s1[:, None] * np.sin(ang1) * valid).astype(np.float32)
    a2re = (s2[:, None] * np.cos(ang2) * valid).astype(np.float32)
    a2im = (-s2[:, None] * np.sin(ang2) * valid).astype(np.float32)
    for ft in range(FT):
        ksl = slice(ft * 128, (ft + 1) * 128)
        a[ft, :, 0:128] = a1re[:, ksl]
        a[ft, :, 128:256] = a1im[:, ksl]
        a[ft, :, 256:384] = a2re[:, ksl]
        a[ft, :, 384:512] = a2im[:, ksl]

    w = np.where(valid, 2.0 / P, 0.0)
    w[0] = 1.0 / P
    w[P // 2] = 1.0 / P
    s_idx = np.arange(64, dtype=np.float64)
    phi = 2 * np.pi * np.outer(k, s_idx) / P
    cphi = (w[:, None] * np.cos(phi)).astype(np.float32).reshape(FT, 128, 64)
    sphi = (w[:, None] * np.sin(phi)).astype(np.float32).reshape(FT, 128, 64)

    km = np.arange(128, dtype=np.float64)
    alpha = 2 * np.pi * np.outer(km, km) / 128
    cosa = np.cos(alpha).astype(np.float32)
    nsina = (-np.sin(alpha)).astype(np.float32)

    xt = np.ascontiguousarray(x.reshape(B * HW, C).T)  # [C, 6272]

    ah, al = _split_fp32r(a)
    xh, xl = _split_fp32r(xt)
    cosah, cosal = _split_fp32r(cosa)
    nsinah, nsinal = _split_fp32r(nsina)
    return ah, al, cphi, sphi, cosah, cosal, nsinah, nsinal, xh, xl


def _round_fp32r(f):
    """RNE to 11 mantissa bits — matches TRN2 fp32r rounding exactly."""
    u = np.ascontiguousarray(f).view(np.uint32)
    drop = 12
    r = u + np.uint32((1 << (drop - 1)) - 1) + ((u >> drop) & np.uint32(1))
    r = (r >> drop) << drop
    return r.view(np.float32)


def _split_fp32r(f):
    hi = _round_fp32r(f)
    lo = _round_fp32r((f - hi).astype(np.float32))
    return hi, lo


def _make_in_maps(x, M1, M2):
    ah, al, cphi, sphi, cosah, cosal, nsinah, nsinal, xh, xl = _host_prep(x, M1, M2)
    in_maps = []
    for r in range(NCORES):
        in_maps.append({
            "ah": ah,
            "al": al,
            "xh": np.ascontiguousarray(xh[:, r * T:(r + 1) * T]),
            "xl": np.ascontiguousarray(xl[:, r * T:(r + 1) * T]),
            "cphi": cphi,
            "sphi": sphi,
            "cosah": cosah,
            "cosal": cosal,
            "nsinah": nsinah,
            "nsinal": nsinal,
        })
    return in_maps


def kernel(x, M1, M2):
    from concourse.bass_utils import run_bass_kernel_spmd

    if "nc" not in _CACHE:
        _CACHE["nc"] = _build_program()
    nc = _CACHE["nc"]

    in_maps = _make_in_maps(x, M1, M2)
    res = run_bass_kernel_spmd(nc, in_maps, core_ids=list(range(NCORES)))
    out = np.concatenate([res.results[r]["y"] for r in range(NCORES)], axis=0)
    return out.astype(np.float32)


# revision 20
# speedup vs baseline: 1.0628x; 1.0628x over previous
"""Trainium2 kernel for CompactBilinearLayer (count-sketch bilinear pooling).

Math: reference computes y = l2norm(signed_sqrt(sum_hw Re IFFT(FFT(x@M1)*FFT(x@M2)))).
Since M1/M2 are count-sketch matrices (one +-1 per row), FFT(x@M1) == x @ A1 with
A1[c,k] = s1[c] * exp(-2pi i h1[c] k / P) — a dense [512, K] matrix computable on the
host from M1 in O(C*K). The IFFT is linear, so the spatial sum moves before it.
Hermitian symmetry means only k = 0..4096 are needed.  Per core (4 batch elements,
784 spatial positions — fully batch-local, no collectives):
  A: P1/P2 projections  = A^T @ x^T          (4 real matmuls, k on partitions)
  B: S[k,b] = sum_t (P1*P2) per batch        (complex product + segmented reduce)
  C: IFFT via two-step factorization n=64q+s: U/V twiddle (DVE) + matmul over k%128
  D: signed sqrt + per-batch L2 norm + store
"""
import numpy as np

P = 8192
C = 512
FT = 33            # frequency tiles of 128 -> 4224 slots >= 4097
NSLOT = FT * 128
NCORES = 8
BPC = 4            # batch elems per core
HW = 196           # spatial positions per batch elem
T = BPC * HW       # 784 positions per core
B = 32

_CACHE = {}


def _build_program():
    import concourse.bass as bass
    import concourse.tile as tile
    from concourse import bacc, mybir

    f32 = mybir.dt.float32
    f32r = mybir.dt.float32r
    nc = bacc.Bacc("TRN2", target_bir_lowering=False, debug=False,
                   num_devices=NCORES)

    # stage-A operands come pre-split (hi/lo, each RNE-rounded to fp32r's
    # 11 mantissa bits on the host); 3 full-rate fp32r matmuls give exact
    # fp32-quality products at 3/4 the PE cost of native fp32 (4 cyc/row).
    ah_d = nc.dram_tensor("ah", [FT, C, 512], f32r, kind="ExternalInput").ap()
    al_d = nc.dram_tensor("al", [FT, C, 512], f32r, kind="ExternalInput").ap()
    xh_d = nc.dram_tensor("xh", [C, T], f32r, kind="ExternalInput").ap()
    xl_d = nc.dram_tensor("xl", [C, T], f32r, kind="ExternalInput").ap()
    cphi_d = nc.dram_tensor("cphi", [FT, 128, 64], f32, kind="ExternalInput").ap()
    sphi_d = nc.dram_tensor("sphi", [FT, 128, 64], f32, kind="ExternalInput").ap()
    cosah_d = nc.dram_tensor("cosah", [128, 128], f32r, kind="ExternalInput").ap()
    cosal_d = nc.dram_tensor("cosal", [128, 128], f32r, kind="ExternalInput").ap()
    nsinah_d = nc.dram_tensor("nsinah", [128, 128], f32r, kind="ExternalInput").ap()
    nsinal_d = nc.dram_tensor("nsinal", [128, 128], f32r, kind="ExternalInput").ap()
    y_d = nc.dram_tensor("y", [BPC, P], f32, kind="ExternalOutput").ap()

    mult = mybir.AluOpType.mult
    add = mybir.AluOpType.add
    Act = mybir.ActivationFunctionType

    with tile.TileContext(nc) as tc:
        with (
            tc.tile_pool(name="const", bufs=1) as const,
            tc.tile_pool(name="apool", bufs=3) as apool,
            tc.tile_pool(name="ps", bufs=1, space="PSUM") as pspool,
            tc.tile_pool(name="scr", bufs=3) as scr,
            tc.tile_pool(name="uv", bufs=3) as uvpool,
        ):
            xh_sb = const.tile([128, 4, T], f32r)
            nc.sync.dma_start(xh_sb[:], xh_d.rearrange("(ck p) t -> p ck t", p=128))
            xl_sb = const.tile([128, 4, T], f32r)
            nc.sync.dma_start(xl_sb[:], xl_d.rearrange("(ck p) t -> p ck t", p=128))
            cphi_sb = const.tile([128, FT, 64], f32)
            nc.sync.dma_start(cphi_sb[:], cphi_d.rearrange("kt p s -> p kt s"))
            sphi_sb = const.tile([128, FT, 64], f32)
            nc.sync.dma_start(sphi_sb[:], sphi_d.rearrange("kt p s -> p kt s"))
            cosah_sb = const.tile([128, 128], f32r)
            nc.sync.dma_start(cosah_sb[:], cosah_d)
            cosal_sb = const.tile([128, 128], f32r)
            nc.sync.dma_start(cosal_sb[:], cosal_d)
            nsinah_sb = const.tile([128, 128], f32r)
            nc.sync.dma_start(nsinah_sb[:], nsinah_d)
            nsinal_sb = const.tile([128, 128], f32r)
            nc.sync.dma_start(nsinal_sb[:], nsinal_d)
            ones_sb = const.tile([128, 1], f32)
            nc.vector.memset(ones_sb[:], 1.0)
            sre_sb = const.tile([128, FT * 4], f32)
            sim_sb = const.tile([128, FT * 4], f32)

            # ---- stage A+B: projections, complex product, spatial reduce ----
            for ft in range(FT):
                ah_t = apool.tile([128, 4, 512], f32r, tag="ah")
                nc.sync.dma_start(
                    ah_t[:], ah_d[ft].rearrange("(ck p) m -> p ck m", p=128)
                )
                al_t = apool.tile([128, 4, 512], f32r, tag="al")
                nc.sync.dma_start(
                    al_t[:], al_d[ft].rearrange("(ck p) m -> p ck m", p=128)
                )
                ps = [
                    pspool.tile([128, T], f32, tag=f"p{m}", name=f"ps{m}_{ft}")
                    for m in range(4)
                ]
                for m in range(4):
                    msl = slice(m * 128, (m + 1) * 128)
                    for ck in range(4):
                        for c0, cn in ((0, 512), (512, T - 512)):
                            terms = (
                                (ah_t[:, ck, msl], xh_sb[:, ck, c0:c0 + cn]),
                                (ah_t[:, ck, msl], xl_sb[:, ck, c0:c0 + cn]),
                                (al_t[:, ck, msl], xh_sb[:, ck, c0:c0 + cn]),
                            )
                            for ti, (lhs, rhs) in enumerate(terms):
                                nc.tensor.matmul(
                                    ps[m][:, c0:c0 + cn],
                                    lhs,
                                    rhs,
                                    start=(ck == 0 and ti == 0),
                                    stop=(ck == 3 and ti == 2),
                                )
                # DVE reads at most one PSUM operand; stage the A2 pair in SBUF
                p2sb = scr.tile([128, T], f32, tag="p2sb")
                p3sb = scr.tile([128, T], f32, tag="p3sb")
                nc.scalar.activation(p2sb[:], ps[2][:], Act.Copy)
                nc.scalar.activation(p3sb[:], ps[3][:], Act.Copy)
                operands = ((ps[0], p2sb), (ps[1], p3sb), (ps[0], p3sb), (ps[1], p2sb))
                red = []
                for i, (pa, pb) in enumerate(operands):
                    prod = scr.tile([128, T], f32, tag=f"prod{i}",
                                    name=f"prod{i}_{ft}")
                    nc.vector.tensor_tensor(prod[:], pa[:], pb[:], op=mult)
                    r = scr.tile([128, BPC], f32, tag=f"red{i}",
                                 name=f"red{i}_{ft}")
                    nc.vector.reduce_sum(
                        out=r[:],
                        in_=prod[:].rearrange("p (b t) -> p b t", b=BPC),
                        axis=mybir.AxisListType.X,
                    )
                    red.append(r)
                sblk = slice(ft * 4, (ft + 1) * 4)
                nc.vector.tensor_sub(sre_sb[:, sblk], red[0][:], red[1][:])
                nc.vector.tensor_add(sim_sb[:, sblk], red[2][:], red[3][:])

            # ---- stage C: twiddle + IFFT matmul over k mod 128 ----
            psy = pspool.tile([128, BPC * 64], f32, tag="p0")
            for kt in range(FT):
                cph = cphi_sb[:, kt, :][:, None, :].broadcast_to([128, BPC, 64])
                sph = sphi_sb[:, kt, :][:, None, :].broadcast_to([128, BPC, 64])
                sre = sre_sb[:, kt * 4:(kt + 1) * 4][:, :, None].broadcast_to(
                    [128, BPC, 64])
                sim = sim_sb[:, kt * 4:(kt + 1) * 4][:, :, None].broadcast_to(
                    [128, BPC, 64])
                u1 = uvpool.tile([128, BPC, 64], f32, tag="u1")
                u2 = uvpool.tile([128, BPC, 64], f32, tag="u2")
                uu = uvpool.tile([128, BPC * 64], f32, tag="uu")
                v1 = uvpool.tile([128, BPC, 64], f32, tag="v1")
                v2 = uvpool.tile([128, BPC, 64], f32, tag="v2")
                vv = uvpool.tile([128, BPC * 64], f32, tag="vv")
                nc.vector.tensor_tensor(u1[:], cph, sre, op=mult)
                nc.vector.tensor_tensor(u2[:], sph, sim, op=mult)
                nc.vector.tensor_sub(
                    uu[:].rearrange("p (b s) -> p b s", b=BPC), u1[:], u2[:])
                nc.vector.tensor_tensor(v1[:], sph, sre, op=mult)
                nc.vector.tensor_tensor(v2[:], cph, sim, op=mult)
                nc.vector.tensor_add(
                    vv[:].rearrange("p (b s) -> p b s", b=BPC), v1[:], v2[:])
                # split U/V into fp32r hi/lo for full-rate matmuls
                uuh = uvpool.tile([128, BPC * 64], f32r, tag="uuh")
                uul32 = uvpool.tile([128, BPC * 64], f32, tag="uul32")
                uul = uvpool.tile([128, BPC * 64], f32r, tag="uul")
                nc.vector.tensor_copy(uuh[:], uu[:])
                nc.vector.tensor_sub(uul32[:], uu[:], uuh[:].bitcast(f32))
                nc.vector.tensor_copy(uul[:], uul32[:])
                vvh = uvpool.tile([128, BPC * 64], f32r, tag="vvh")
                vvl32 = uvpool.tile([128, BPC * 64], f32, tag="vvl32")
                vvl = uvpool.tile([128, BPC * 64], f32r, tag="vvl")
                nc.vector.tensor_copy(vvh[:], vv[:])
                nc.vector.tensor_sub(vvl32[:], vv[:], vvh[:].bitcast(f32))
                nc.vector.tensor_copy(vvl[:], vvl32[:])
                for ti, (lhs, rhs) in enumerate((
                    (cosah_sb, uuh), (cosah_sb, uul), (cosal_sb, uuh),
                    (nsinah_sb, vvh), (nsinah_sb, vvl), (nsinal_sb, vvh),
                )):
                    nc.tensor.matmul(psy[:], lhs[:], rhs[:],
                                     start=(kt == 0 and ti == 0),
                                     stop=(kt == FT - 1 and ti == 5))

            # ---- stage D: signed sqrt, per-batch l2 norm, store ----
            absy = scr.tile([128, BPC * 64], f32, tag="absy")
            nc.scalar.activation(absy[:], psy[:], Act.Abs)
            sqy = scr.tile([128, BPC * 64], f32, tag="sqy")
            nc.scalar.activation(sqy[:], absy[:], Act.Sqrt)
            sgn = scr.tile([128, BPC * 64], f32, tag="sgn")
            nc.scalar.activation(sgn[:], psy[:], Act.Sign)
            ys = scr.tile([128, BPC * 64], f32, tag="ys")
            nc.vector.tensor_mul(ys[:], sqy[:], sgn[:])

            psn = pspool.tile([128, BPC * 64], f32, tag="p1")
            nc.tensor.matmul(psn[0:1, :], ones_sb[:], absy[:],
                             start=True, stop=True)
            nsq = scr.tile([1, BPC], f32, tag="nsq")
            nc.vector.reduce_sum(
                out=nsq[:],
                in_=psn[0:1, :].rearrange("p (b s) -> p b s", b=BPC),
                axis=mybir.AxisListType.X,
            )
            nc.vector.tensor_scalar_max(nsq[:], nsq[:], 1e-10)
            sqn = scr.tile([1, BPC], f32, tag="sqn")
            nc.scalar.activation(sqn[:], nsq[:], Act.Sqrt)
            invn = scr.tile([1, BPC], f32, tag="invn")
            nc.vector.reciprocal(invn[:], sqn[:])

            onesrow = const.tile([1, 128], f32)
            nc.vector.memset(onesrow[:], 1.0)
            psb = pspool.tile([128, BPC * 64], f32, tag="p2")
            nc.tensor.matmul(psb[:, 0:BPC], onesrow[0:1, :], invn[0:1, :],
                             start=True, stop=True)
            inv_b = psb[:, 0:BPC][:, :, None].broadcast_to([128, BPC, 64])
            fin = scr.tile([128, BPC * 64], f32, tag="fin")
            nc.vector.tensor_tensor(
                fin[:].rearrange("p (b s) -> p b s", b=BPC),
                ys[:].rearrange("p (b s) -> p b s", b=BPC),
                inv_b,
                op=mult,
            )
            for b in range(BPC):
                nc.sync.dma_start(
                    y_d[b].rearrange("(q s) -> q s", q=128),
                    fin[:, b * 64:(b + 1) * 64],
                )

    nc.compile()
    return nc


def _host_prep(x, M1, M2):
    x = np.ascontiguousarray(np.asarray(x, np.float32))
    M1 = np.asarray(M1, np.float32)
    M2 = np.asarray(M2, np.float32)

    h1 = np.argmax(np.abs(M1), axis=1)
    s1 = M1[np.arange(C), h1].astype(np.float64)
    h2 = np.argmax(np.abs(M2), axis=1)
    s2 = M2[np.arange(C), h2].astype(np.float64)

    k = np.arange(NSLOT, dtype=np.float64)
    valid = k <= P // 2
    ang1 = 2 * np.pi * np.outer(h1.astype(np.float64), k) / P
    ang2 = 2 * np.pi * np.outer(h2.astype(np.float64), k) / P
    # a[ft, c, m*128 + j]: m in (A1re, A1im, A2re, A2im), freq = ft*128 + j
    a = np.empty((FT, C, 512), np.float32)
    a1re = (s1[:, None] * np.cos(ang1) * valid).astype(np.float32)
    a1im = (-s1[:, None] * np.sin(ang1) * valid).astype(np.float32)
    a2re = (s2[:, None] * np.cos(ang2) * valid).astype(np.float32)
    a2im = (-s2[:, None] * np.sin(ang2) * valid).astype(np.float32)
    for ft in range(FT):
        ksl = slice(ft * 128, (ft + 1) * 128)
        a[ft, :, 0:128] = a1re[:, ksl]
        a[ft, :, 128:256] = a1im[:, ksl]
        a[ft, :, 256:384] = a2re[:, ksl]
        a[ft, :, 384:512] = a2im[:, ksl]

    w = np.where(valid, 2.0 / P, 0.0)
    w[0] = 1.0 / P
    w[P // 2] = 1.0 / P
    s_idx = np.arange(64, dtype=np.float64)
    phi = 2 * np.pi * np.outer(k, s_idx) / P
    cphi = (w[:, None] * np.cos(phi)).astype(np.float32).reshape(FT, 128, 64)
    sphi = (w[:, None] * np.sin(phi)).astype(np.float32).reshape(FT, 128, 64)

    km = np.arange(128, dtype=np.float64)
    alpha = 2 * np.pi * np.outer(km, km) / 128
    cosa = np.cos(alpha).astype(np.float32)
    nsina = (-np.sin(alpha)).astype(np.float32)

    xt = np.ascontiguousarray(x.reshape(B * HW, C).T)  # [C, 6272]

    ah, al = _split_fp32r(a)
    xh, xl = _split_fp32r(xt)
    cosah, cosal = _split_fp32r(cosa)
    nsinah, nsinal = _split_fp32r(nsina)
    return ah, al, cphi, sphi, cosah, cosal, nsinah, nsinal, xh, xl


def _round_fp32r(f):
    """RNE to 11 mantissa bits — matches TRN2 fp32r rounding exactly."""
    u = np.ascontiguousarray(f).view(np.uint32)
    drop = 12
    r = u + np.uint32((1 << (drop - 1)) - 1) + ((u >> drop) & np.uint32(1))
    r = (r >> drop) << drop
    return r.view(np.float32)


def _split_fp32r(f):
    hi = _round_fp32r(f)
    lo = _round_fp32r((f - hi).astype(np.float32))
    return hi, lo


def _make_in_maps(x, M1, M2):
    ah, al, cphi, sphi, cosah, cosal, nsinah, nsinal, xh, xl = _host_prep(x, M1, M2)
    in_maps = []
    for r in range(NCORES):
        in_maps.append({
            "ah": ah,
            "al": al,
            "xh": np.ascontiguousarray(xh[:, r * T:(r + 1) * T]),
            "xl": np.ascontiguousarray(xl[:, r * T:(r + 1) * T]),
            "cphi": cphi,
            "sphi": sphi,
            "cosah": cosah,
            "cosal": cosal,
            "nsinah": nsinah,
            "nsinal": nsinal,
        })
    return in_maps


def kernel(x, M1, M2):
    from concourse.bass_utils import run_bass_kernel_spmd

    if "nc" not in _CACHE:
        _CACHE["nc"] = _build_program()
    nc = _CACHE["nc"]

    in_maps = _make_in_maps(x, M1, M2)
    res = run_bass_kernel_spmd(nc, in_maps, core_ids=list(range(NCORES)))
    out = np.concatenate([res.results[r]["y"] for r in range(NCORES)], axis=0)
    return out.astype(np.float32)


# revision 22
# speedup vs baseline: 1.1287x; 1.0620x over previous
"""Trainium2 kernel for CompactBilinearLayer (count-sketch bilinear pooling).

Math: reference computes y = l2norm(signed_sqrt(sum_hw Re IFFT(FFT(x@M1)*FFT(x@M2)))).
Since M1/M2 are count-sketch matrices (one +-1 per row), FFT(x@M1) == x @ A1 with
A1[c,k] = s1[c] * exp(-2pi i h1[c] k / P) — a dense [512, K] matrix computable on the
host from M1 in O(C*K). The IFFT is linear, so the spatial sum moves before it.
Hermitian symmetry means only k = 0..4096 are needed.  Per core (4 batch elements,
784 spatial positions — fully batch-local, no collectives):
  A: P1/P2 projections  = A^T @ x^T     (split-fp32r: hi/lo RNE-11 parts, 3
     full-rate matmuls == exact fp32 quality at 3/4 the PE cost of native fp32)
  B: S[k,b] = sum_t (P1*P2) per batch   (complex product + segmented reduce)
  C: IFFT via two-step factorization n=64q+s: U/V twiddle (DVE) + matmul over k%128
  D: signed sqrt + per-batch L2 norm + store
"""
import numpy as np

P = 8192
C = 512
FT = 33            # frequency tiles of 128 -> 4224 slots >= 4097
NSLOT = FT * 128
NCORES = 8
BPC = 4            # batch elems per core
HW = 196           # spatial positions per batch elem
T = BPC * HW       # 784 positions per core
B = 32

_CACHE = {}


def _build_program():
    import concourse.bass as bass
    import concourse.tile as tile
    from concourse import bacc, mybir

    f32 = mybir.dt.float32
    f32r = mybir.dt.float32r
    nc = bacc.Bacc("TRN2", target_bir_lowering=False, debug=False,
                   num_devices=NCORES)

    ah_d = nc.dram_tensor("ah", [FT, C, 512], f32r, kind="ExternalInput").ap()
    al_d = nc.dram_tensor("al", [FT, C, 512], f32r, kind="ExternalInput").ap()
    xh_d = nc.dram_tensor("xh", [C, T], f32r, kind="ExternalInput").ap()
    xl_d = nc.dram_tensor("xl", [C, T], f32r, kind="ExternalInput").ap()
    cphi_d = nc.dram_tensor("cphi", [FT, 128, 64], f32, kind="ExternalInput").ap()
    sphi_d = nc.dram_tensor("sphi", [FT, 128, 64], f32, kind="ExternalInput").ap()
    cosa_d = nc.dram_tensor("cosa", [128, 128], f32, kind="ExternalInput").ap()
    nsina_d = nc.dram_tensor("nsina", [128, 128], f32, kind="ExternalInput").ap()
    y_d = nc.dram_tensor("y", [BPC, P], f32, kind="ExternalOutput").ap()

    mult = mybir.AluOpType.mult
    Act = mybir.ActivationFunctionType

    with tile.TileContext(nc) as tc:
        with (
            tc.tile_pool(name="const", bufs=1) as const,
            tc.tile_pool(name="apool", bufs=3) as apool,
            tc.tile_pool(name="ps", bufs=1, space="PSUM") as pspool,
            tc.tile_pool(name="scr", bufs=3) as scr,
            tc.tile_pool(name="uv", bufs=3) as uvpool,
        ):
            xh_sb = const.tile([128, 4, T], f32r)
            nc.sync.dma_start(xh_sb[:], xh_d.rearrange("(ck p) t -> p ck t", p=128))
            xl_sb = const.tile([128, 4, T], f32r)
            nc.sync.dma_start(xl_sb[:], xl_d.rearrange("(ck p) t -> p ck t", p=128))
            cphi_sb = const.tile([128, FT, 64], f32)
            nc.sync.dma_start(cphi_sb[:], cphi_d.rearrange("kt p s -> p kt s"))
            sphi_sb = const.tile([128, FT, 64], f32)
            nc.sync.dma_start(sphi_sb[:], sphi_d.rearrange("kt p s -> p kt s"))
            cosa_sb = const.tile([128, 128], f32)
            nc.sync.dma_start(cosa_sb[:], cosa_d)
            nsina_sb = const.tile([128, 128], f32)
            nc.sync.dma_start(nsina_sb[:], nsina_d)
            ones_sb = const.tile([128, 1], f32)
            nc.vector.memset(ones_sb[:], 1.0)
            sre_sb = const.tile([128, FT * 4], f32)
            sim_sb = const.tile([128, FT * 4], f32)

            # ---- stage A+B: projections, complex product, spatial reduce ----
            for ft in range(FT):
                ah_t = apool.tile([128, 4, 512], f32r, tag="ah")
                nc.sync.dma_start(
                    ah_t[:], ah_d[ft].rearrange("(ck p) m -> p ck m", p=128)
                )
                al_t = apool.tile([128, 4, 512], f32r, tag="al")
                nc.sync.dma_start(
                    al_t[:], al_d[ft].rearrange("(ck p) m -> p ck m", p=128)
                )
                ps = [
                    pspool.tile([128, T], f32, tag=f"p{m}", name=f"ps{m}_{ft}")
                    for m in range(4)
                ]
                for m in range(4):
                    msl = slice(m * 128, (m + 1) * 128)
                    for ck in range(4):
                        for c0, cn in ((0, 512), (512, T - 512)):
                            terms = (
                                (ah_t[:, ck, msl], xh_sb[:, ck, c0:c0 + cn]),
                                (ah_t[:, ck, msl], xl_sb[:, ck, c0:c0 + cn]),
                                (al_t[:, ck, msl], xh_sb[:, ck, c0:c0 + cn]),
                            )
                            for ti, (lhs, rhs) in enumerate(terms):
                                nc.tensor.matmul(
                                    ps[m][:, c0:c0 + cn],
                                    lhs,
                                    rhs,
                                    start=(ck == 0 and ti == 0),
                                    stop=(ck == 3 and ti == 2),
                                )
                # DVE reads at most one PSUM operand; stage the A2 pair in SBUF
                p2sb = scr.tile([128, T], f32, tag="p2sb")
                p3sb = scr.tile([128, T], f32, tag="p3sb")
                nc.scalar.activation(p2sb[:], ps[2][:], Act.Copy)
                nc.scalar.activation(p3sb[:], ps[3][:], Act.Copy)
                operands = ((ps[0], p2sb), (ps[1], p3sb), (ps[0], p3sb), (ps[1], p2sb))
                red = []
                for i, (pa, pb) in enumerate(operands):
                    prod = scr.tile([128, T], f32, tag=f"prod{i}",
                                    name=f"prod{i}_{ft}")
                    nc.vector.tensor_tensor(prod[:], pa[:], pb[:], op=mult)
                    r = scr.tile([128, BPC], f32, tag=f"red{i}",
                                 name=f"red{i}_{ft}")
                    nc.vector.reduce_sum(
                        out=r[:],
                        in_=prod[:].rearrange("p (b t) -> p b t", b=BPC),
                        axis=mybir.AxisListType.X,
                    )
                    red.append(r)
                sblk = slice(ft * 4, (ft + 1) * 4)
                nc.vector.tensor_sub(sre_sb[:, sblk], red[0][:], red[1][:])
                nc.vector.tensor_add(sim_sb[:, sblk], red[2][:], red[3][:])

            # ---- stage C: twiddle + IFFT matmul over k mod 128 ----
            psy = pspool.tile([128, BPC * 64], f32, tag="p0")
            for kt in range(FT):
                cph = cphi_sb[:, kt, :][:, None, :].broadcast_to([128, BPC, 64])
                sph = sphi_sb[:, kt, :][:, None, :].broadcast_to([128, BPC, 64])
                sre = sre_sb[:, kt * 4:(kt + 1) * 4][:, :, None].broadcast_to(
                    [128, BPC, 64])
                sim = sim_sb[:, kt * 4:(kt + 1) * 4][:, :, None].broadcast_to(
                    [128, BPC, 64])
                u1 = uvpool.tile([128, BPC, 64], f32, tag="u1")
                u2 = uvpool.tile([128, BPC, 64], f32, tag="u2")
                uu = uvpool.tile([128, BPC * 64], f32, tag="uu")
                v1 = uvpool.tile([128, BPC, 64], f32, tag="v1")
                v2 = uvpool.tile([128, BPC, 64], f32, tag="v2")
                vv = uvpool.tile([128, BPC * 64], f32, tag="vv")
                nc.vector.tensor_tensor(u1[:], cph, sre, op=mult)
                nc.vector.tensor_tensor(u2[:], sph, sim, op=mult)
                nc.vector.tensor_sub(
                    uu[:].rearrange("p (b s) -> p b s", b=BPC), u1[:], u2[:])
                nc.vector.tensor_tensor(v1[:], sph, sre, op=mult)
                nc.vector.tensor_tensor(v2[:], cph, sim, op=mult)
                nc.vector.tensor_add(
                    vv[:].rearrange("p (b s) -> p b s", b=BPC), v1[:], v2[:])
                nc.tensor.matmul(psy[:], cosa_sb[:], uu[:],
                                 start=(kt == 0), stop=False)
                nc.tensor.matmul(psy[:], nsina_sb[:], vv[:],
                                 start=False, stop=(kt == FT - 1))

            # ---- stage D: signed sqrt, per-batch l2 norm, store ----
            absy = scr.tile([128, BPC * 64], f32, tag="absy")
            nc.scalar.activation(absy[:], psy[:], Act.Abs)
            sqy = scr.tile([128, BPC * 64], f32, tag="sqy")
            nc.scalar.activation(sqy[:], absy[:], Act.Sqrt)
            sgn = scr.tile([128, BPC * 64], f32, tag="sgn")
            nc.scalar.activation(sgn[:], psy[:], Act.Sign)
            ys = scr.tile([128, BPC * 64], f32, tag="ys")
            nc.vector.tensor_mul(ys[:], sqy[:], sgn[:])

            psn = pspool.tile([128, BPC * 64], f32, tag="p1")
            nc.tensor.matmul(psn[0:1, :], ones_sb[:], absy[:],
                             start=True, stop=True)
            nsq = scr.tile([1, BPC], f32, tag="nsq")
            nc.vector.reduce_sum(
                out=nsq[:],
                in_=psn[0:1, :].rearrange("p (b s) -> p b s", b=BPC),
                axis=mybir.AxisListType.X,
            )
            nc.vector.tensor_scalar_max(nsq[:], nsq[:], 1e-10)
            sqn = scr.tile([1, BPC], f32, tag="sqn")
            nc.scalar.activation(sqn[:], nsq[:], Act.Sqrt)
            invn = scr.tile([1, BPC], f32, tag="invn")
            nc.vector.reciprocal(invn[:], sqn[:])

            onesrow = const.tile([1, 128], f32)
            nc.vector.memset(onesrow[:], 1.0)
            psb = pspool.tile([128, BPC * 64], f32, tag="p2")
            nc.tensor.matmul(psb[:, 0:BPC], onesrow[0:1, :], invn[0:1, :],
                             start=True, stop=True)
            inv_b = psb[:, 0:BPC][:, :, None].broadcast_to([128, BPC, 64])
            fin = scr.tile([128, BPC * 64], f32, tag="fin")
            nc.vector.tensor_tensor(
                fin[:].rearrange("p (b s) -> p b s", b=BPC),
                ys[:].rearrange("p (b s) -> p b s", b=BPC),
                inv_b,
                op=mult,
            )
            for b in range(BPC):
                nc.sync.dma_start(
                    y_d[b].rearrange("(q s) -> q s", q=128),
                    fin[:, b * 64:(b + 1) * 64],
                )

    nc.compile()
    return nc


def _round_fp32r(f):
    """RNE to 11 mantissa bits — matches TRN2 fp32r rounding exactly."""
    u = np.ascontiguousarray(f).view(np.uint32)
    drop = 12
    r = u + np.uint32((1 << (drop - 1)) - 1) + ((u >> drop) & np.uint32(1))
    r = (r >> drop) << drop
    return r.view(np.float32)


def _split_fp32r(f):
    hi = _round_fp32r(f)
    lo = _round_fp32r((f - hi).astype(np.float32))
    return hi, lo


def _host_prep(x, M1, M2):
    x = np.ascontiguousarray(np.asarray(x, np.float32))
    M1 = np.asarray(M1, np.float32)
    M2 = np.asarray(M2, np.float32)

    h1 = np.argmax(np.abs(M1), axis=1)
    s1 = M1[np.arange(C), h1].astype(np.float64)
    h2 = np.argmax(np.abs(M2), axis=1)
    s2 = M2[np.arange(C), h2].astype(np.float64)

    k = np.arange(NSLOT, dtype=np.float64)
    valid = k <= P // 2
    ang1 = 2 * np.pi * np.outer(h1.astype(np.float64), k) / P
    ang2 = 2 * np.pi * np.outer(h2.astype(np.float64), k) / P
    # a[ft, c, m*128 + j]: m in (A1re, A1im, A2re, A2im), freq = ft*128 + j
    a = np.empty((FT, C, 512), np.float32)
    a1re = (s1[:, None] * np.cos(ang1) * valid).astype(np.float32)
    a1im = (-s1[:, None] * np.sin(ang1) * valid).astype(np.float32)
    a2re = (s2[:, None] * np.cos(ang2) * valid).astype(np.float32)
    a2im = (-s2[:, None] * np.sin(ang2) * valid).astype(np.float32)
    for ft in range(FT):
        ksl = slice(ft * 128, (ft + 1) * 128)
        a[ft, :, 0:128] = a1re[:, ksl]
        a[ft, :, 128:256] = a1im[:, ksl]
        a[ft, :, 256:384] = a2re[:, ksl]
        a[ft, :, 384:512] = a2im[:, ksl]

    w = np.where(valid, 2.0 / P, 0.0)
    w[0] = 1.0 / P
    w[P // 2] = 1.0 / P
    s_idx = np.arange(64, dtype=np.float64)
    phi = 2 * np.pi * np.outer(k, s_idx) / P
    cphi = (w[:, None] * np.cos(phi)).astype(np.float32).reshape(FT, 128, 64)
    sphi = (w[:, None] * np.sin(phi)).astype(np.float32).reshape(FT, 128, 64)

    km = np.arange(128, dtype=np.float64)
    alpha = 2 * np.pi * np.outer(km, km) / 128
    cosa = np.cos(alpha).astype(np.float32)
    nsina = (-np.sin(alpha)).astype(np.float32)

    xt = np.ascontiguousarray(x.reshape(B * HW, C).T)  # [C, 6272]

    ah, al = _split_fp32r(a)
    xh, xl = _split_fp32r(xt)
    return ah, al, cphi, sphi, cosa, nsina, xh, xl


def _make_in_maps(x, M1, M2):
    ah, al, cphi, sphi, cosa, nsina, xh, xl = _host_prep(x, M1, M2)
    in_maps = []
    for r in range(NCORES):
        in_maps.append({
            "ah": ah,
            "al": al,
            "xh": np.ascontiguousarray(xh[:, r * T:(r + 1) * T]),
            "xl": np.ascontiguousarray(xl[:, r * T:(r + 1) * T]),
            "cphi": cphi,
            "sphi": sphi,
            "cosa": cosa,
            "nsina": nsina,
        })
    return in_maps


def kernel(x, M1, M2):
    from concourse.bass_utils import run_bass_kernel_spmd

    if "nc" not in _CACHE:
        _CACHE["nc"] = _build_program()
    nc = _CACHE["nc"]

    in_maps = _make_in_maps(x, M1, M2)
    res = run_bass_kernel_spmd(nc, in_maps, core_ids=list(range(NCORES)))
    out = np.concatenate([res.results[r]["y"] for r in range(NCORES)], axis=0)
    return out.astype(np.float32)


# revision 24
# speedup vs baseline: 1.1325x; 1.0034x over previous
"""Trainium2 kernel for CompactBilinearLayer (count-sketch bilinear pooling).

Math: reference computes y = l2norm(signed_sqrt(sum_hw Re IFFT(FFT(x@M1)*FFT(x@M2)))).
Since M1/M2 are count-sketch matrices (one +-1 per row), FFT(x@M1) == x @ A1 with
A1[c,k] = s1[c] * exp(-2pi i h1[c] k / P) — a dense [512, K] matrix computable on the
host from M1 in O(C*K). The IFFT is linear, so the spatial sum moves before it.
Hermitian symmetry means only k = 0..4096 are needed.  Per core (4 batch elements,
784 spatial positions — fully batch-local, no collectives):
  A: P1/P2 projections  = A^T @ x^T     (split-fp32r: hi/lo RNE-11 parts, 3
     full-rate matmuls == exact fp32 quality at 3/4 the PE cost of native fp32)
  B: S[k,b] = sum_t (P1*P2) per batch   (complex product + segmented reduce)
  C: IFFT via two-step factorization n=64q+s: U/V twiddle (DVE) + matmul over k%128
  D: signed sqrt + per-batch L2 norm + store
"""
import numpy as np

P = 8192
C = 512
FT = 33            # frequency tiles of 128 -> 4224 slots >= 4097
NSLOT = FT * 128
NCORES = 8
BPC = 4            # batch elems per core
HW = 196           # spatial positions per batch elem
T = BPC * HW       # 784 positions per core
B = 32

_CACHE = {}


def _build_program():
    import concourse.bass as bass
    import concourse.tile as tile
    from concourse import bacc, mybir

    f32 = mybir.dt.float32
    f32r = mybir.dt.float32r
    nc = bacc.Bacc("TRN2", target_bir_lowering=False, debug=False,
                   num_devices=NCORES)

    ah_d = nc.dram_tensor("ah", [FT, C, 512], f32r, kind="ExternalInput").ap()
    al_d = nc.dram_tensor("al", [FT, C, 512], f32r, kind="ExternalInput").ap()
    xh_d = nc.dram_tensor("xh", [C, T], f32r, kind="ExternalInput").ap()
    xl_d = nc.dram_tensor("xl", [C, T], f32r, kind="ExternalInput").ap()
    cphi_d = nc.dram_tensor("cphi", [FT, 128, 64], f32, kind="ExternalInput").ap()
    sphi_d = nc.dram_tensor("sphi", [FT, 128, 64], f32, kind="ExternalInput").ap()
    cosa_d = nc.dram_tensor("cosa", [128, 128], f32, kind="ExternalInput").ap()
    nsina_d = nc.dram_tensor("nsina", [128, 128], f32, kind="ExternalInput").ap()
    y_d = nc.dram_tensor("y", [BPC, P], f32, kind="ExternalOutput").ap()

    mult = mybir.AluOpType.mult
    Act = mybir.ActivationFunctionType

    with tile.TileContext(nc) as tc:
        with (
            tc.tile_pool(name="const", bufs=1) as const,
            tc.tile_pool(name="apool", bufs=3) as apool,
            tc.tile_pool(name="ps", bufs=1, space="PSUM") as pspool,
            tc.tile_pool(name="scr", bufs=3) as scr,
            tc.tile_pool(name="uv", bufs=4) as uvpool,
        ):
            xh_sb = const.tile([128, 4, T], f32r)
            nc.sync.dma_start(xh_sb[:], xh_d.rearrange("(ck p) t -> p ck t", p=128))
            xl_sb = const.tile([128, 4, T], f32r)
            nc.sync.dma_start(xl_sb[:], xl_d.rearrange("(ck p) t -> p ck t", p=128))
            cphi_sb = const.tile([128, FT, 64], f32)
            nc.sync.dma_start(cphi_sb[:], cphi_d.rearrange("kt p s -> p kt s"))
            sphi_sb = const.tile([128, FT, 64], f32)
            nc.sync.dma_start(sphi_sb[:], sphi_d.rearrange("kt p s -> p kt s"))
            cosa_sb = const.tile([128, 128], f32)
            nc.sync.dma_start(cosa_sb[:], cosa_d)
            nsina_sb = const.tile([128, 128], f32)
            nc.sync.dma_start(nsina_sb[:], nsina_d)
            ones_sb = const.tile([128, 1], f32)
            nc.vector.memset(ones_sb[:], 1.0)
            sre_sb = const.tile([128, FT * 4], f32)
            sim_sb = const.tile([128, FT * 4], f32)

            # ---- stage A+B: projections, complex product, spatial reduce ----
            for ft in range(FT):
                ah_t = apool.tile([128, 4, 512], f32r, tag="ah")
                nc.sync.dma_start(
                    ah_t[:], ah_d[ft].rearrange("(ck p) m -> p ck m", p=128)
                )
                al_t = apool.tile([128, 4, 512], f32r, tag="al")
                nc.sync.dma_start(
                    al_t[:], al_d[ft].rearrange("(ck p) m -> p ck m", p=128)
                )
                ps = [
                    pspool.tile([128, T], f32, tag=f"p{m}", name=f"ps{m}_{ft}")
                    for m in range(4)
                ]
                for m in range(4):
                    msl = slice(m * 128, (m + 1) * 128)
                    for ck in range(4):
                        for c0, cn in ((0, 512), (512, T - 512)):
                            terms = (
                                (ah_t[:, ck, msl], xh_sb[:, ck, c0:c0 + cn]),
                                (ah_t[:, ck, msl], xl_sb[:, ck, c0:c0 + cn]),
                                (al_t[:, ck, msl], xh_sb[:, ck, c0:c0 + cn]),
                            )
                            for ti, (lhs, rhs) in enumerate(terms):
                                nc.tensor.matmul(
                                    ps[m][:, c0:c0 + cn],
                                    lhs,
                                    rhs,
                                    start=(ck == 0 and ti == 0),
                                    stop=(ck == 3 and ti == 2),
                                )
                # DVE reads at most one PSUM operand; stage the A2 pair in SBUF
                p2sb = scr.tile([128, T], f32, tag="p2sb")
                p3sb = scr.tile([128, T], f32, tag="p3sb")
                nc.scalar.activation(p2sb[:], ps[2][:], Act.Copy)
                nc.scalar.activation(p3sb[:], ps[3][:], Act.Copy)
                operands = ((ps[0], p2sb), (ps[1], p3sb), (ps[0], p3sb), (ps[1], p2sb))
                red = []
                for i, (pa, pb) in enumerate(operands):
                    prod = scr.tile([128, T], f32, tag=f"prod{i}",
                                    name=f"prod{i}_{ft}")
                    nc.vector.tensor_tensor(prod[:], pa[:], pb[:], op=mult)
                    r = scr.tile([128, BPC], f32, tag=f"red{i}",
                                 name=f"red{i}_{ft}")
                    nc.vector.reduce_sum(
                        out=r[:],
                        in_=prod[:].rearrange("p (b t) -> p b t", b=BPC),
                        axis=mybir.AxisListType.X,
                    )
                    red.append(r)
                sblk = slice(ft * 4, (ft + 1) * 4)
                nc.vector.tensor_sub(sre_sb[:, sblk], red[0][:], red[1][:])
                nc.vector.tensor_add(sim_sb[:, sblk], red[2][:], red[3][:])

            # ---- stage C: twiddle + IFFT matmul over k mod 128 ----
            psy = pspool.tile([128, BPC * 64], f32, tag="p0")
            for kt in range(FT):
                cph = cphi_sb[:, kt, :][:, None, :].broadcast_to([128, BPC, 64])
                sph = sphi_sb[:, kt, :][:, None, :].broadcast_to([128, BPC, 64])
                sre = sre_sb[:, kt * 4:(kt + 1) * 4][:, :, None].broadcast_to(
                    [128, BPC, 64])
                sim = sim_sb[:, kt * 4:(kt + 1) * 4][:, :, None].broadcast_to(
                    [128, BPC, 64])
                u1 = uvpool.tile([128, BPC, 64], f32, tag="u1")
                u2 = uvpool.tile([128, BPC, 64], f32, tag="u2")
                uu = uvpool.tile([128, BPC * 64], f32, tag="uu")
                v1 = uvpool.tile([128, BPC, 64], f32, tag="v1")
                v2 = uvpool.tile([128, BPC, 64], f32, tag="v2")
                vv = uvpool.tile([128, BPC * 64], f32, tag="vv")
                nc.vector.tensor_tensor(u1[:], cph, sre, op=mult)
                nc.vector.tensor_tensor(u2[:], sph, sim, op=mult)
                nc.vector.tensor_sub(
                    uu[:].rearrange("p (b s) -> p b s", b=BPC), u1[:], u2[:])
                nc.vector.tensor_tensor(v1[:], sph, sre, op=mult)
                nc.vector.tensor_tensor(v2[:], cph, sim, op=mult)
                nc.vector.tensor_add(
                    vv[:].rearrange("p (b s) -> p b s", b=BPC), v1[:], v2[:])
                nc.tensor.matmul(psy[:], cosa_sb[:], uu[:],
                                 start=(kt == 0), stop=False)
                nc.tensor.matmul(psy[:], nsina_sb[:], vv[:],
                                 start=False, stop=(kt == FT - 1))

            # ---- stage D: signed sqrt, per-batch l2 norm, store ----
            absy = scr.tile([128, BPC * 64], f32, tag="absy")
            nc.scalar.activation(absy[:], psy[:], Act.Abs)
            sqy = scr.tile([128, BPC * 64], f32, tag="sqy")
            nc.scalar.activation(sqy[:], absy[:], Act.Sqrt)
            sgn = scr.tile([128, BPC * 64], f32, tag="sgn")
            nc.scalar.activation(sgn[:], psy[:], Act.Sign)
            ys = scr.tile([128, BPC * 64], f32, tag="ys")
            nc.vector.tensor_mul(ys[:], sqy[:], sgn[:])

            psn = pspool.tile([128, BPC * 64], f32, tag="p1")
            nc.tensor.matmul(psn[0:1, :], ones_sb[:], absy[:],
                             start=True, stop=True)
            nsq = scr.tile([1, BPC], f32, tag="nsq")
            nc.vector.reduce_sum(
                out=nsq[:],
                in_=psn[0:1, :].rearrange("p (b s) -> p b s", b=BPC),
                axis=mybir.AxisListType.X,
            )
            nc.vector.tensor_scalar_max(nsq[:], nsq[:], 1e-10)
            sqn = scr.tile([1, BPC], f32, tag="sqn")
            nc.scalar.activation(sqn[:], nsq[:], Act.Sqrt)
            invn = scr.tile([1, BPC], f32, tag="invn")
            nc.vector.reciprocal(invn[:], sqn[:])

            onesrow = const.tile([1, 128], f32)
            nc.vector.memset(onesrow[:], 1.0)
            psb = pspool.tile([128, BPC * 64], f32, tag="p2")
            nc.tensor.matmul(psb[:, 0:BPC], onesrow[0:1, :], invn[0:1, :],
                             start=True, stop=True)
            inv_b = psb[:, 0:BPC][:, :, None].broadcast_to([128, BPC, 64])
            fin = scr.tile([128, BPC * 64], f32, tag="fin")
            nc.vector.tensor_tensor(
                fin[:].rearrange("p (b s) -> p b s", b=BPC),
                ys[:].rearrange("p (b s) -> p b s", b=BPC),
                inv_b,
                op=mult,
            )
            for b in range(BPC):
                nc.sync.dma_start(
                    y_d[b].rearrange("(q s) -> q s", q=128),
                    fin[:, b * 64:(b + 1) * 64],
                )

    nc.compile()
    return nc


def _round_fp32r(f):
    """RNE to 11 mantissa bits — matches TRN2 fp32r rounding exactly."""
    u = np.ascontiguousarray(f).view(np.uint32)
    drop = 12
    r = u + np.uint32((1 << (drop - 1)) - 1) + ((u >> drop) & np.uint32(1))
    r = (r >> drop) << drop
    return r.view(np.float32)


def _split_fp32r(f):
    hi = _round_fp32r(f)
    lo = _round_fp32r((f - hi).astype(np.float32))
    return hi, lo


def _host_prep(x, M1, M2):
    x = np.ascontiguousarray(np.asarray(x, np.float32))
    M1 = np.asarray(M1, np.float32)
    M2 = np.asarray(M2, np.float32)

    h1 = np.argmax(np.abs(M1), axis=1)
    s1 = M1[np.arange(C), h1].astype(np.float64)
    h2 = np.argmax(np.abs(M2), axis=1)
    s2 = M2[np.arange(C), h2].astype(np.float64)

    k = np.arange(NSLOT, dtype=np.float64)
    valid = k <= P // 2
    ang1 = 2 * np.pi * np.outer(h1.astype(np.float64), k) / P
    ang2 = 2 * np.pi * np.outer(h2.astype(np.float64), k) / P
    # a[ft, c, m*128 + j]: m in (A1re, A1im, A2re, A2im), freq = ft*128 + j
    a = np.empty((FT, C, 512), np.float32)
    a1re = (s1[:, None] * np.cos(ang1) * valid).astype(np.float32)
    a1im = (-s1[:, None] * np.sin(ang1) * valid).astype(np.float32)
    a2re = (s2[:, None] * np.cos(ang2) * valid).astype(np.float32)
    a2im = (-s2[:, None] * np.sin(ang2) * valid).astype(np.float32)
    for ft in range(FT):
        ksl = slice(ft * 128, (ft + 1) * 128)
        a[ft, :, 0:128] = a1re[:, ksl]
        a[ft, :, 128:256] = a1im[:, ksl]
        a[ft, :, 256:384] = a2re[:, ksl]
        a[ft, :, 384:512] = a2im[:, ksl]

    w = np.where(valid, 2.0 / P, 0.0)
    w[0] = 1.0 / P
    w[P // 2] = 1.0 / P
    s_idx = np.arange(64, dtype=np.float64)
    phi = 2 * np.pi * np.outer(k, s_idx) / P
    cphi = (w[:, None] * np.cos(phi)).astype(np.float32).reshape(FT, 128, 64)
    sphi = (w[:, None] * np.sin(phi)).astype(np.float32).reshape(FT, 128, 64)

    km = np.arange(128, dtype=np.float64)
    alpha = 2 * np.pi * np.outer(km, km) / 128
    cosa = np.cos(alpha).astype(np.float32)
    nsina = (-np.sin(alpha)).astype(np.float32)

    xt = np.ascontiguousarray(x.reshape(B * HW, C).T)  # [C, 6272]

    ah, al = _split_fp32r(a)
    xh, xl = _split_fp32r(xt)
    return ah, al, cphi, sphi, cosa, nsina, xh, xl


def _make_in_maps(x, M1, M2):
    ah, al, cphi, sphi, cosa, nsina, xh, xl = _host_prep(x, M1, M2)
    in_maps = []
    for r in range(NCORES):
        in_maps.append({
            "ah": ah,
            "al": al,
            "xh": np.ascontiguousarray(xh[:, r * T:(r + 1) * T]),
            "xl": np.ascontiguousarray(xl[:, r * T:(r + 1) * T]),
            "cphi": cphi,
            "sphi": sphi,
            "cosa": cosa,
            "nsina": nsina,
        })
    return in_maps


def kernel(x, M1, M2):
    from concourse.bass_utils import run_bass_kernel_spmd

    if "nc" not in _CACHE:
        _CACHE["nc"] = _build_program()
    nc = _CACHE["nc"]

    in_maps = _make_in_maps(x, M1, M2)
    res = run_bass_kernel_spmd(nc, in_maps, core_ids=list(range(NCORES)))
    out = np.concatenate([res.results[r]["y"] for r in range(NCORES)], axis=0)
    return out.astype(np.float32)
